# revision 1
# baseline (speedup 1.0000x reference)
"""Trainium2 Bass kernel for nn_CausalSelfAttention_43009802502282.

Causal self-attention with external memory (mem_k == mem_v), B=4, T=2048,
C=1024, 16 heads x 64, MEM=256.

Sharding (8 cores): core c -> batch b = c//2, head-group g = c%2 (8 heads).
Each core computes qkv for its heads (column-sliced W_attn), attention for
its 8 heads, and a partial projection (row-sliced W_proj). Host sums the
two bf16 partials per batch (f32) and adds b_proj.

v2: all matmul inputs bf16 (f32r blocked LDWEIGHTS double-buffering and ran
the PE at ~half rate), QKV projections for all chunks emitted up-front and
chunk ch+1's projection interleaved behind chunk ch's attention so the PE
never waits on the softmax-denominator chain. Denominators: av psum ones
column -> SBUF bf16 -> row DMAs into one [8,512] tile per chunk -> single
DVE reciprocal -> PE broadcast via a 0/1 selection matmul -> one [128,512]
DVE multiply per pair. Causal mask added only on the 128-wide diagonal
block via a bf16 identity matmul. Bias for q/k fused into the ScalarE
psum->SBUF copy (Identity activation with per-partition bias).
"""
import sys

sys.path.insert(0, "/opt/trn_rl_repo")

import numpy as np
import ml_dtypes
import concourse.bass as bass
import concourse.mybir as mybir
from concourse import bacc
from concourse.tile import TileContext
from concourse.bass_utils import run_bass_kernel_spmd

F32 = mybir.dt.float32
BF16 = mybir.dt.bfloat16
EXP = mybir.ActivationFunctionType.Exp
IDENT = mybir.ActivationFunctionType.Identity
NPBF16 = ml_dtypes.bfloat16

B, T, C = 4, 2048, 1024
NH, D, MEM = 16, 64, 256
HL = 8                        # heads per core
P = 128
S = MEM + T                   # 2304
NST = S // P                  # 18 s-tiles (0,1 = memory; 2..17 causal)
NKT = C // P                  # 8 contraction tiles
NEG = -1.0e30
SCALE = 0.125                 # 1/sqrt(64)
LAG = 2

_CACHE: dict = {}


def _build(num_devices=8, reps=1):
    nc = bacc.Bacc("TRN2", target_bir_lowering=False, debug=False, num_devices=num_devices)

    xT_d = nc.dram_tensor("xT", [C, T], BF16, kind="ExternalInput")
    wqk_d = nc.dram_tensor("wqk", [C, 1024], BF16, kind="ExternalInput")
    wv_d = nc.dram_tensor("wv", [C, 512], BF16, kind="ExternalInput")
    wp_d = nc.dram_tensor("wp", [512, C], BF16, kind="ExternalInput")
    memT_d = nc.dram_tensor("memT", [HL * D, MEM], BF16, kind="ExternalInput")
    memn_d = nc.dram_tensor("memn", [MEM, HL * D], BF16, kind="ExternalInput")
    bqk_d = nc.dram_tensor("bqk", [P, 8], F32, kind="ExternalInput")
    bv_d = nc.dram_tensor("bv", [1, 512], BF16, kind="ExternalInput")
    onesr_d = nc.dram_tensor("onesr", [1, P], BF16, kind="ExternalInput")
    mask_d = nc.dram_tensor("mask", [P, P], BF16, kind="ExternalInput")
    bsel_d = nc.dram_tensor("bsel", [8, 4 * P], BF16, kind="ExternalInput")
    out_d = nc.dram_tensor("out", [T, C], BF16, kind="ExternalOutput")

    import contextlib

    with TileContext(nc) as tc:
        with (tc.For_i(0, reps, 1) if reps > 1 else contextlib.nullcontext()):
            _body(nc, tc, dict(locals()))
    nc.compile()
    return nc


def _body(nc, tc, dr):
    xT_d, wqk_d, wv_d, wp_d = dr["xT_d"], dr["wqk_d"], dr["wv_d"], dr["wp_d"]
    memT_d, memn_d = dr["memT_d"], dr["memn_d"]
    bqk_d, bv_d, onesr_d = dr["bqk_d"], dr["bv_d"], dr["onesr_d"]
    mask_d, bsel_d, out_d = dr["mask_d"], dr["bsel_d"], dr["out_d"]

    with tc.tile_pool(name="pers", bufs=1) as pers, \
         tc.tile_pool(name="expp", bufs=4) as expp, \
         tc.tile_pool(name="stgp", bufs=4) as stgp, \
         tc.tile_pool(name="stkp", bufs=9) as stkp, \
         tc.tile_pool(name="densp", bufs=4) as densp, \
         tc.tile_pool(name="otp", bufs=3) as otp, \
         tc.tile_pool(name="ps_s", bufs=2, space="PSUM") as ps_s, \
         tc.tile_pool(name="scps", bufs=2, space="PSUM") as scps, \
         tc.tile_pool(name="avps", bufs=2, space="PSUM") as avps:

        wqk_s = pers.tile([P, NKT, 1024], BF16, tag="wqk")
        wv_s = pers.tile([P, NKT, 512], BF16, tag="wv")
        wp_s = pers.tile([P, 4, C], BF16, tag="wp")
        xT_s = pers.tile([P, NKT, T], BF16, tag="xT")
        qkT = pers.tile([P, 8, T], BF16, tag="qkT")    # rows: q tiles 0-3, k tiles 4-7
        v_s = pers.tile([P, NST, HL, D + 1], BF16, tag="v")
        memT_s = pers.tile([P, 4, MEM], BF16, tag="memT")
        mask_s = pers.tile([P, P], BF16, tag="mask")
        bqk_s = pers.tile([P, 8], F32, tag="bqk")
        bv_s = pers.tile([1, 512], BF16, tag="bv")
        onesr_s = pers.tile([1, P], BF16, tag="onesr")
        bsel_s = pers.tile([8, 4, P], BF16, tag="bsel")

        # ---- input loads, ordered so the first projection starts ASAP:
        # wqk/wv on sync; xT split per (c, chunk) with chunk 0 first,
        # alternating gpsimd/scalar; small tensors early on scalar ----
        nc.scalar.dma_start(bqk_s[:], bqk_d[:, :])
        nc.scalar.dma_start(bv_s[:], bv_d[:, :])
        nc.scalar.dma_start(onesr_s[:], onesr_d[:, :])
        for c in range(NKT):
            nc.sync.dma_start(wqk_s[:, c], wqk_d[P * c:P * c + P, :])
        for ch in range(4):
            for c in range(NKT):
                eng = nc.gpsimd if c % 2 == 0 else nc.scalar
                eng.dma_start(xT_s[:, c, 512 * ch:512 * ch + 512],
                              xT_d[P * c:P * c + P, 512 * ch:512 * ch + 512])
        for c in range(NKT):
            nc.sync.dma_start(wv_s[:, c], wv_d[P * c:P * c + P, :])
        nc.sync.dma_start(memT_s[:], memT_d[:, :].rearrange("(rt p) m -> p rt m", p=P))
        for st in range(2):
            nc.gpsimd.dma_start(v_s[:, st, :, 0:D],
                                memn_d[P * st:P * st + P, :]
                                .rearrange("p (j d) -> p j d", d=D))
        nc.vector.memset(v_s[:, :, :, D], 1.0)
        nc.scalar.dma_start(mask_s[:], mask_d[:, :])
        nc.sync.dma_start(bsel_s[:], bsel_d[:, :].rearrange("j (pr m) -> j pr m", m=P))
        nc.gpsimd.dma_start(wp_s[:], wp_d[:, :].rearrange("(rt p) c -> p rt c", p=P))

        def projB(ch):
            # qkv projections for tokens [512ch, 512ch+512)
            for mt in range(8):
                ps = ps_s.tile([P, 512], F32, tag="ps", name=f"qk_{ch}_{mt}")
                for c in range(NKT):
                    nc.tensor.matmul(ps[:], wqk_s[:, c, P * mt:P * mt + P],
                                     xT_s[:, c, 512 * ch:512 * ch + 512],
                                     start=(c == 0), stop=(c == NKT - 1))
                nc.vector.tensor_scalar_add(qkT[:, mt, 512 * ch:512 * ch + 512], ps[:],
                                            bqk_s[:, mt:mt + 1])
            for tl in range(4):
                st = 2 + 4 * ch + tl
                ps = ps_s.tile([P, 512], F32, tag="ps", name=f"v_{ch}_{tl}")
                for c in range(NKT):
                    nc.tensor.matmul(ps[:], xT_s[:, c, 512 * ch + P * tl:512 * ch + P * tl + P],
                                     wv_s[:, c], start=(c == 0), stop=False)
                nc.tensor.matmul(ps[:], onesr_s[:], bv_s[:], start=False, stop=True)
                nc.vector.tensor_copy(v_s[:, st, :, 0:D], ps[:])

        def proj_out(ch, stks):
            for mt in range(4):
                for n2 in range(2):
                    pp = ps_s.tile([P, 512], F32, tag="ps", name=f"pj_{ch}_{mt}_{n2}")
                    for pr in range(4):
                        nc.tensor.matmul(pp[:], stks[pr][:, P * mt:P * mt + P],
                                         wp_s[:, pr, 512 * n2:512 * n2 + 512],
                                         start=(pr == 0), stop=(pr == 3))
                    ot = otp.tile([P, 512], BF16, tag="ot", name=f"ot_{ch}_{mt}_{n2}")
                    nc.vector.tensor_copy(ot[:], pp[:])
                    nc.gpsimd.dma_start(
                        out_d[512 * ch + P * mt:512 * ch + P * mt + P,
                              512 * n2:512 * n2 + 512], ot[:])

        projB(0)
        projB(1)
        chunk_stks = {}

        for ch in range(4):
            n_st = 6 + 4 * ch
            stks = []
            dens = densp.tile([8, 512], BF16, tag="dens", name=f"dens_{ch}")
            for pr in range(4):
                j0, j1 = 2 * pr, 2 * pr + 1
                qT0 = qkT[0:64, pr, 512 * ch:512 * ch + 512]
                qT1 = qkT[64:128, pr, 512 * ch:512 * ch + 512]
                av0 = avps.tile([D + 1, 512], F32, tag="av", name=f"av0_{ch}_{pr}")
                av1 = avps.tile([D + 1, 512], F32, tag="av", name=f"av1_{ch}_{pr}")

                def emit_av(st, ex):
                    vs = P * (st - 2 - 4 * ch) if st >= 2 + 4 * ch else 0
                    nc.tensor.matmul(av0[:, vs:512], v_s[:, st, j0], ex[:, vs:512],
                                     start=(st == 0), stop=(st == n_st - 1))
                    nc.tensor.matmul(av1[:, vs:512], v_s[:, st, j1], ex[:, 512 + vs:1024],
                                     start=(st == 0), stop=(st == n_st - 1))

                pend = []
                for st in range(n_st):
                    diag = st >= 2 + 4 * ch
                    if st < 2:
                        kf0 = memT_s[0:64, pr, P * st:P * st + P]
                        kf1 = memT_s[64:128, pr, P * st:P * st + P]
                    else:
                        kf0 = qkT[0:64, 4 + pr, P * (st - 2):P * (st - 2) + P]
                        kf1 = qkT[64:128, 4 + pr, P * (st - 2):P * (st - 2) + P]
                    sp = st - 2 - 4 * ch if diag else 0
                    vs = P * sp
                    sc = scps.tile([P, 1024], F32, tag="sc", name=f"sc_{ch}_{pr}_{st}")
                    nc.tensor.matmul(sc[:, vs:512], kf0, qT0[:, vs:512], start=True, stop=True)
                    nc.tensor.matmul(sc[:, 512 + vs:1024], kf1, qT1[:, vs:512],
                                     start=True, stop=True)
                    ex = expp.tile([P, 1024], BF16, tag="ex", name=f"ex_{ch}_{pr}_{st}")
                    if vs == 0:
                        nc.scalar.activation(ex[:], sc[:], EXP, scale=SCALE)
                    else:
                        nc.scalar.activation(ex[:, vs:512], sc[:, vs:512], EXP, scale=SCALE)
                        nc.scalar.activation(ex[:, 512 + vs:1024], sc[:, 512 + vs:1024],
                                             EXP, scale=SCALE)
                    if diag:
                        # zero the causally-invalid upper triangle of the
                        # 128-wide diagonal block (0/1 mask, DVE)
                        nc.vector.tensor_tensor(ex[:, vs:vs + P], ex[:, vs:vs + P],
                                                mask_s[:], mybir.AluOpType.mult)
                        nc.vector.tensor_tensor(ex[:, 512 + vs:512 + vs + P],
                                                ex[:, 512 + vs:512 + vs + P],
                                                mask_s[:], mybir.AluOpType.mult)
                    pend.append((st, ex))
                    if len(pend) > LAG:
                        emit_av(*pend.pop(0))
                for st_ex in pend:
                    emit_av(*st_ex)

                # numerators + denominator row to SBUF (bf16), then row DMAs
                stg0 = stgp.tile([D + 1, 512], BF16, tag="stg", name=f"stg0_{ch}_{pr}")
                stg1 = stgp.tile([D + 1, 512], BF16, tag="stg", name=f"stg1_{ch}_{pr}")
                nc.vector.tensor_copy(stg0[:], av0[:])
                nc.vector.tensor_copy(stg1[:], av1[:])
                stk = stkp.tile([P, 512], BF16, tag="stk", name=f"stk_{ch}_{pr}")
                nc.sync.dma_start(dens[j0:j0 + 1, :], stg0[D:D + 1, :])
                nc.sync.dma_start(dens[j1:j1 + 1, :], stg1[D:D + 1, :])
                nc.sync.dma_start(stk[0:D, :], stg0[0:D, :])
                nc.sync.dma_start(stk[D:2 * D, :], stg1[0:D, :])
                stks.append(stk)

            # reciprocal of all 8 denominators at once (off the PE path)
            rden = densp.tile([8, 512], BF16, tag="dens", name=f"rden_{ch}")
            with nc.allow_low_precision(reason="softmax denom reciprocal"):
                nc.vector.reciprocal(rden[:], dens[:])

            # keep the PE busy while the denominator chain (row DMAs +
            # reciprocal) completes: next-next chunk's projections, or the
            # previous chunk's output projection
            if ch < 2:
                projB(ch + 2)
            elif ch == 2:
                proj_out(1, chunk_stks[1])

            # broadcast 1/den to 128 rows per pair via selection matmul, then
            # scale the stacked numerators in place
            for pr in range(4):
                bc = ps_s.tile([P, 512], F32, tag="ps", name=f"bc_{ch}_{pr}")
                nc.tensor.matmul(bc[:], bsel_s[:, pr], rden[:], start=True, stop=True)
                nc.vector.tensor_tensor(stks[pr][:], stks[pr][:], bc[:],
                                        mybir.AluOpType.mult)
            chunk_stks[ch] = stks
            if ch == 1:
                proj_out(0, chunk_stks[0])
            elif ch == 3:
                proj_out(2, chunk_stks[2])
                proj_out(3, chunk_stks[3])


def _host_inputs(x, ext_mem, W_attn, b_attn, W_proj, b_proj):
    """Per-core input maps (host-side sharding/layout prep, no FLOPs)."""
    mask = np.where(np.arange(P)[:, None] <= np.arange(P)[None, :], 1.0, 0.0)
    onesr = np.ones((1, P), dtype=np.float32)
    bsel = np.zeros((8, 4, P), dtype=np.float32)
    for pr in range(4):
        bsel[2 * pr, pr, 0:64] = 1.0
        bsel[2 * pr + 1, pr, 64:128] = 1.0

    bf = lambda a: np.ascontiguousarray(a).astype(NPBF16)
    in_maps = []
    for c in range(8):
        b, g = c // 2, c % 2
        qs = slice(512 * g, 512 * g + 512)
        ks = slice(1024 + 512 * g, 1024 + 512 * g + 512)
        vs = slice(2048 + 512 * g, 2048 + 512 * g + 512)
        mem = np.ascontiguousarray(ext_mem[b][:, 512 * g:512 * g + 512])
        bqk_full = np.concatenate([b_attn[qs], b_attn[ks]]).astype(np.float32)
        in_maps.append({
            "xT": bf(x[b].T),
            "wqk": bf(np.concatenate([W_attn[:, qs], W_attn[:, ks]], axis=1)),
            "wv": bf(W_attn[:, vs]),
            "wp": bf(W_proj[512 * g:512 * g + 512, :]),
            "memT": bf(mem.T),
            "memn": bf(mem),
            "bqk": np.ascontiguousarray(bqk_full.reshape(8, P).T),
            "bv": bf(b_attn[vs][None, :]),
            "onesr": bf(onesr),
            "mask": bf(mask),
            "bsel": bf(bsel.reshape(8, 4 * P)),
        })
    return in_maps


def kernel(x, ext_mem, W_attn, b_attn, W_proj, b_proj):
    x = np.asarray(x, dtype=np.float32)
    ext_mem = np.asarray(ext_mem, dtype=np.float32)
    W_attn = np.asarray(W_attn, dtype=np.float32)
    b_attn = np.asarray(b_attn, dtype=np.float32)
    W_proj = np.asarray(W_proj, dtype=np.float32)
    b_proj = np.asarray(b_proj, dtype=np.float32)

    if "nc" not in _CACHE:
        _CACHE["nc"] = _build()
    nc = _CACHE["nc"]

    in_maps = _host_inputs(x, ext_mem, W_attn, b_attn, W_proj, b_proj)
    res = run_bass_kernel_spmd(nc, in_maps, list(range(8)))

    out = np.empty((B, T, C), dtype=np.float32)
    for b in range(B):
        out[b] = (res.results[2 * b]["out"].astype(np.float32)
                  + res.results[2 * b + 1]["out"].astype(np.float32) + b_proj)
    return out



# revision 3
# speedup vs baseline: 1.5212x; 1.5212x over previous
"""Trainium2 Bass kernel for nn_CausalSelfAttention_43009802502282 (v6, current).

Causal self-attention with external memory (mem_k == mem_v), B=4, T=2048,
C=1024, 16 heads x 64, MEM=256.

Sharding (8 cores): core c -> batch b = c//2, head-group g = c%2 (8 heads).

v4: the attention inner loop is ACT-bound (exp eviction ~775ns/s-tile vs PE
~640ns with row-tiled score pairs concurrent), so QKV/output-projection
matmuls are fed into the s-tile loops ONE AT A TIME from a generator (v3's
8-MM lumps made PE and ACT stall alternately). Diagonal-tile exp pairs are
merged into one activation via a [2, 512-vs] free AP. The v-bias matmul is
gone (memory values de-biased on host; +b_v folds through W_proj into the
host-side output bias). PE warmup matmuls run during the initial DMA loads.
"""
import sys

sys.path.insert(0, "/opt/trn_rl_repo")

import numpy as np
import ml_dtypes
import concourse.bass as bass
import concourse.mybir as mybir
from concourse import bacc
from concourse.tile import TileContext
from concourse.bass_utils import run_bass_kernel_spmd

F32 = mybir.dt.float32
BF16 = mybir.dt.bfloat16
EXP = mybir.ActivationFunctionType.Exp
NPBF16 = ml_dtypes.bfloat16

B, T, C = 4, 2048, 1024
NH, D, MEM = 16, 64, 256
HL = 8                        # heads per core
P = 128
S = MEM + T                   # 2304
NST = S // P                  # 18 s-tiles (0,1 = memory; 2..17 causal)
NKT = C // P                  # 8 contraction tiles
SCALE = 0.125                 # 1/sqrt(64)
LAG = 2
NWARM = 20
FEED = 2                      # proj matmuls fed per attention s-tile group

_CACHE: dict = {}


def _build(num_devices=8, reps=1):
    nc = bacc.Bacc("TRN2", target_bir_lowering=False, debug=False, num_devices=num_devices)

    xT_d = nc.dram_tensor("xT", [C, T], BF16, kind="ExternalInput")
    wqk_d = nc.dram_tensor("wqk", [C, 1024], BF16, kind="ExternalInput")
    wv_d = nc.dram_tensor("wv", [C, 512], BF16, kind="ExternalInput")
    wp_d = nc.dram_tensor("wp", [512, C], BF16, kind="ExternalInput")
    memT_d = nc.dram_tensor("memT", [HL * D, MEM], BF16, kind="ExternalInput")
    memn_d = nc.dram_tensor("memn", [MEM, HL * D], BF16, kind="ExternalInput")
    bqk_d = nc.dram_tensor("bqk", [P, 8], F32, kind="ExternalInput")
    mask_d = nc.dram_tensor("mask", [P, P], BF16, kind="ExternalInput")
    bsel_d = nc.dram_tensor("bsel", [8, 4 * P], BF16, kind="ExternalInput")
    out_d = nc.dram_tensor("out", [T, C], BF16, kind="ExternalOutput")

    import contextlib

    with TileContext(nc) as tc:
        with (tc.For_i(0, reps, 1) if reps > 1 else contextlib.nullcontext()):
            _body(nc, tc, dict(locals()))
    nc.compile()
    return nc


def _body(nc, tc, dr):
    xT_d, wqk_d, wv_d, wp_d = dr["xT_d"], dr["wqk_d"], dr["wv_d"], dr["wp_d"]
    memT_d, memn_d = dr["memT_d"], dr["memn_d"]
    bqk_d = dr["bqk_d"]
    mask_d, bsel_d, out_d = dr["mask_d"], dr["bsel_d"], dr["out_d"]

    with tc.tile_pool(name="pers", bufs=1) as pers, \
         tc.tile_pool(name="expp", bufs=4) as expp, \
         tc.tile_pool(name="stgp", bufs=4) as stgp, \
         tc.tile_pool(name="stkp", bufs=12) as stkp, \
         tc.tile_pool(name="densp", bufs=4) as densp, \
         tc.tile_pool(name="otp", bufs=3) as otp, \
         tc.tile_pool(name="ps_s", bufs=2, space="PSUM") as ps_s, \
         tc.tile_pool(name="scps", bufs=2, space="PSUM") as scps, \
         tc.tile_pool(name="avps", bufs=2, space="PSUM") as avps:

        wqk_s = pers.tile([P, NKT, 1024], BF16, tag="wqk")
        wv_s = pers.tile([P, NKT, 512], BF16, tag="wv")
        wp_s = pers.tile([P, 4, C], BF16, tag="wp")
        xT_s = pers.tile([P, NKT, T], BF16, tag="xT")
        qkT = pers.tile([P, 8, T], BF16, tag="qkT")    # rows: q tiles 0-3, k tiles 4-7
        v_s = pers.tile([P, NST, HL, D + 1], BF16, tag="v")
        memT_s = pers.tile([P, 4, MEM], BF16, tag="memT")
        mask_s = pers.tile([P, P], BF16, tag="mask")
        bqk_s = pers.tile([P, 8], F32, tag="bqk")
        bsel_s = pers.tile([8, 4, P], BF16, tag="bsel")
        scr = pers.tile([P, 512], BF16, tag="scr")

        # ---- input loads, ordered so the first projection starts ASAP ----
        nc.scalar.dma_start(bqk_s[:], bqk_d[:, :])
        for c in range(NKT):
            nc.sync.dma_start(wqk_s[:, c], wqk_d[P * c:P * c + P, :])
        for ch in range(4):
            for c in range(NKT):
                eng = nc.gpsimd if c % 2 == 0 else nc.scalar
                eng.dma_start(xT_s[:, c, 512 * ch:512 * ch + 512],
                              xT_d[P * c:P * c + P, 512 * ch:512 * ch + 512])
        for c in range(NKT):
            nc.sync.dma_start(wv_s[:, c], wv_d[P * c:P * c + P, :])
        nc.sync.dma_start(memT_s[:], memT_d[:, :].rearrange("(rt p) m -> p rt m", p=P))
        for st in range(2):
            nc.gpsimd.dma_start(v_s[:, st, :, 0:D],
                                memn_d[P * st:P * st + P, :]
                                .rearrange("p (j d) -> p j d", d=D))
        nc.vector.memset(v_s[:, :, :, D], 1.0)
        nc.scalar.dma_start(mask_s[:], mask_d[:, :])
        nc.sync.dma_start(bsel_s[:], bsel_d[:, :].rearrange("j (pr m) -> j pr m", m=P))
        nc.gpsimd.dma_start(wp_s[:], wp_d[:, :].rearrange("(rt p) c -> p rt c", p=P))

        # ---- PE warmup: keep HAM/pstate hot while DMAs land ----
        nc.vector.memset(scr[:], 0.125)
        for i in range(NWARM):
            wps = ps_s.tile([P, 512], F32, tag="ps", name=f"warm{i}")
            nc.tensor.matmul(wps[:], scr[:, 0:P], scr[:], start=True, stop=True)

        # ---- projection micro-generators: yield once per matmul ----
        def qk_gen(ch, mt):
            ps = ps_s.tile([P, 512], F32, tag="ps", name=f"qk_{ch}_{mt}")
            for c in range(NKT):
                nc.tensor.matmul(ps[:], wqk_s[:, c, P * mt:P * mt + P],
                                 xT_s[:, c, 512 * ch:512 * ch + 512],
                                 start=(c == 0), stop=(c == NKT - 1))
                yield
            nc.vector.tensor_scalar_add(qkT[:, mt, 512 * ch:512 * ch + 512], ps[:],
                                        bqk_s[:, mt:mt + 1])

        def v_gen(ch, tl):
            st = 2 + 4 * ch + tl
            ps = ps_s.tile([P, 512], F32, tag="ps", name=f"v_{ch}_{tl}")
            for c in range(NKT):
                nc.tensor.matmul(ps[:], xT_s[:, c, 512 * ch + P * tl:512 * ch + P * tl + P],
                                 wv_s[:, c], start=(c == 0), stop=(c == NKT - 1))
                yield
            nc.vector.tensor_copy(v_s[:, st, :, 0:D], ps[:])

        chunk_stks = {}

        def po_gen(ch, mt, n2):
            stks = chunk_stks[ch]
            pp = ps_s.tile([P, 512], F32, tag="ps", name=f"pj_{ch}_{mt}_{n2}")
            for pr in range(4):
                nc.tensor.matmul(pp[:], stks[pr][:, P * mt:P * mt + P],
                                 wp_s[:, pr, 512 * n2:512 * n2 + 512],
                                 start=(pr == 0), stop=(pr == 3))
                yield
            ot = otp.tile([P, 512], BF16, tag="ot", name=f"ot_{ch}_{mt}_{n2}")
            nc.vector.tensor_copy(ot[:], pp[:])
            nc.gpsimd.dma_start(
                out_d[512 * ch + P * mt:512 * ch + P * mt + P,
                      512 * n2:512 * n2 + 512], ot[:])

        def chain(gens):
            for g in gens:
                yield from g

        def projB_gen(ch):
            return chain([qk_gen(ch, mt) for mt in range(8)]
                         + [v_gen(ch, tl) for tl in range(4)])

        def po_gens(ch):
            return chain([po_gen(ch, mt, n2) for mt in range(4) for n2 in range(2)])

        def drain(gen):
            for _ in gen:
                pass

        # projB(0) must complete before attention(0)
        drain(projB_gen(0))

        # rolling feed of proj matmuls; refilled as chunks complete.
        # deadline(ch) = generators that must be exhausted before attention(ch)
        feed = {1: projB_gen(1), 2: projB_gen(2), 3: projB_gen(3)}
        po_feed = []   # appended after each chunk's bc

        for ch in range(4):
            n_st = 6 + 4 * ch
            stks = []
            dens = densp.tile([8, 512], BF16, tag="dens", name=f"dens_{ch}")

            rr = [0]

            def step_feed(k=FEED):
                # alternate between next-chunk projB and pending output-proj
                for _ in range(k):
                    rr[0] ^= 1
                    order = [0, 1] if rr[0] else [1, 0]
                    for which in order:
                        if which == 0 and (ch + 1) in feed:
                            if next(feed[ch + 1], StopIteration) is StopIteration:
                                del feed[ch + 1]
                                continue
                            break
                        if which == 1 and po_feed:
                            if next(po_feed[0], StopIteration) is StopIteration:
                                po_feed.pop(0)
                                continue
                            break

            for pr in range(4):
                j0, j1 = 2 * pr, 2 * pr + 1
                qT0 = qkT[0:64, pr, 512 * ch:512 * ch + 512]
                qT1 = qkT[64:128, pr, 512 * ch:512 * ch + 512]
                av0 = avps.tile([D + 1, 512], F32, tag="av", name=f"av0_{ch}_{pr}")
                av1 = avps.tile([D + 1, 512], F32, tag="av", name=f"av1_{ch}_{pr}")

                def emit_av(st, ex):
                    vs = P * (st - 2 - 4 * ch) if st >= 2 + 4 * ch else 0
                    nc.tensor.matmul(av0[:, vs:512], v_s[:, st, j0], ex[:, vs:512],
                                     start=(st == 0), stop=(st == n_st - 1))
                    nc.tensor.matmul(av1[:, vs:512], v_s[:, st, j1], ex[:, 512 + vs:1024],
                                     start=(st == 0), stop=(st == n_st - 1))

                def emit_scores(st):
                    diag = st >= 2 + 4 * ch
                    if st < 2:
                        kf0 = memT_s[0:64, pr, P * st:P * st + P]
                        kf1 = memT_s[64:128, pr, P * st:P * st + P]
                    else:
                        kf0 = qkT[0:64, 4 + pr, P * (st - 2):P * (st - 2) + P]
                        kf1 = qkT[64:128, 4 + pr, P * (st - 2):P * (st - 2) + P]
                    sp = st - 2 - 4 * ch if diag else 0
                    vs = P * sp
                    sc = scps.tile([P, 1024], F32, tag="sc", name=f"sc_{ch}_{pr}_{st}")
                    nc.tensor.matmul(sc[:, vs:512], kf0, qT0[:, vs:512], start=True, stop=True)
                    nc.tensor.matmul(sc[:, 512 + vs:1024], kf1, qT1[:, vs:512],
                                     start=True, stop=True)
                    return sc, vs, diag

                def emit_exp(st, sc, vs, diag):
                    ex = expp.tile([P, 1024], BF16, tag="ex", name=f"ex_{ch}_{pr}_{st}")
                    if vs == 0:
                        nc.scalar.activation(ex[:], sc[:], EXP, scale=SCALE)
                    else:
                        # both heads' valid ranges in one activation:
                        # free AP [2 heads, 512-vs] with stride 512 between heads
                        sc3 = sc[:].rearrange("p (h q) -> p h q", h=2)[:, :, vs:512]
                        ex3 = ex[:].rearrange("p (h q) -> p h q", h=2)[:, :, vs:512]
                        nc.scalar.activation(ex3, sc3, EXP, scale=SCALE)
                    if diag:
                        # zero the causally-invalid upper triangle of the
                        # 128-wide diagonal block (0/1 mask, DVE)
                        nc.vector.tensor_tensor(ex[:, vs:vs + P], ex[:, vs:vs + P],
                                                mask_s[:], mybir.AluOpType.mult)
                        nc.vector.tensor_tensor(ex[:, 512 + vs:512 + vs + P],
                                                ex[:, 512 + vs:512 + vs + P],
                                                mask_s[:], mybir.AluOpType.mult)
                    return ex

                # st-pairs: both scores pairs (64x128 tile mode) back-to-back,
                # then both delayed AVs + feeds (128x128 mode) -- halves the
                # PE tile-mode switches, each of which drains the array
                pend = []
                for stp in range(0, n_st, 2):
                    r0 = emit_scores(stp)
                    r1 = emit_scores(stp + 1)
                    pend.append((stp, emit_exp(stp, *r0)))
                    pend.append((stp + 1, emit_exp(stp + 1, *r1)))
                    while len(pend) > LAG:
                        emit_av(*pend.pop(0))
                    step_feed()
                    step_feed()
                for st_ex in pend:
                    emit_av(*st_ex)

                # numerators + denominator row to SBUF (bf16), then row DMAs
                stg0 = stgp.tile([D + 1, 512], BF16, tag="stg", name=f"stg0_{ch}_{pr}")
                stg1 = stgp.tile([D + 1, 512], BF16, tag="stg", name=f"stg1_{ch}_{pr}")
                nc.vector.tensor_copy(stg0[:], av0[:])
                nc.vector.tensor_copy(stg1[:], av1[:])
                stk = stkp.tile([P, 512], BF16, tag="stk", name=f"stk_{ch}_{pr}")
                nc.sync.dma_start(dens[j0:j0 + 1, :], stg0[D:D + 1, :])
                nc.sync.dma_start(dens[j1:j1 + 1, :], stg1[D:D + 1, :])
                nc.sync.dma_start(stk[0:D, :], stg0[0:D, :])
                nc.sync.dma_start(stk[D:2 * D, :], stg1[0:D, :])
                stks.append(stk)
                step_feed(2)

            # before attention(ch+1): its projB must be complete
            if ch + 1 in feed:
                drain(feed.pop(ch + 1))

            # reciprocal of all 8 denominators at once (off the PE path)
            rden = densp.tile([8, 512], BF16, tag="dens", name=f"rden_{ch}")
            with nc.allow_low_precision(reason="softmax denom reciprocal"):
                nc.vector.reciprocal(rden[:], dens[:])

            # older chunks' output projections must finish before this chunk's
            # bc (bounds stk-tile liveness to two chunks); also covers the
            # reciprocal/DMA latency with PE work
            while po_feed:
                drain(po_feed.pop(0))

            # broadcast 1/den to 128 rows per pair via selection matmul, then
            # scale the stacked numerators in place
            for pr in range(4):
                bc = ps_s.tile([P, 512], F32, tag="ps", name=f"bc_{ch}_{pr}")
                nc.tensor.matmul(bc[:], bsel_s[:, pr], rden[:], start=True, stop=True)
                nc.vector.tensor_tensor(stks[pr][:], stks[pr][:], bc[:],
                                        mybir.AluOpType.mult)
            chunk_stks[ch] = stks
            po_feed.append(po_gens(ch))

        for g in po_feed:
            drain(g)


def _host_inputs(x, ext_mem, W_attn, b_attn, W_proj, b_proj):
    """Per-core input maps (host-side sharding/layout prep, no heavy FLOPs)."""
    mask = np.where(np.arange(P)[:, None] <= np.arange(P)[None, :], 1.0, 0.0)
    bsel = np.zeros((8, 4, P), dtype=np.float32)
    for pr in range(4):
        bsel[2 * pr, pr, 0:64] = 1.0
        bsel[2 * pr + 1, pr, 64:128] = 1.0

    bf = lambda a: np.ascontiguousarray(a).astype(NPBF16)
    in_maps = []
    for c in range(8):
        b, g = c // 2, c % 2
        qs = slice(512 * g, 512 * g + 512)
        ks = slice(1024 + 512 * g, 1024 + 512 * g + 512)
        mem = np.ascontiguousarray(ext_mem[b][:, 512 * g:512 * g + 512])
        bqk_full = np.concatenate([b_attn[qs], b_attn[ks]]).astype(np.float32)
        bv = b_attn[2048 + 512 * g:2048 + 512 * g + 512].astype(np.float32)
        in_maps.append({
            "xT": bf(x[b].T),
            "wqk": bf(np.concatenate([W_attn[:, qs], W_attn[:, ks]], axis=1)),
            "wv": bf(W_attn[:, 2048 + 512 * g:2048 + 512 * g + 512]),
            "wp": bf(W_proj[512 * g:512 * g + 512, :]),
            "memT": bf(mem.T),
            # v-side memory values are de-biased so the uniform +b_v can fold
            # through the projection into the host-side output bias
            "memn": bf(mem - bv[None, :]),
            "bqk": np.ascontiguousarray(bqk_full.reshape(8, P).T),
            "mask": bf(mask),
            "bsel": bf(bsel.reshape(8, 4 * P)),
        })
    return in_maps


def kernel(x, ext_mem, W_attn, b_attn, W_proj, b_proj):
    x = np.asarray(x, dtype=np.float32)
    ext_mem = np.asarray(ext_mem, dtype=np.float32)
    W_attn = np.asarray(W_attn, dtype=np.float32)
    b_attn = np.asarray(b_attn, dtype=np.float32)
    W_proj = np.asarray(W_proj, dtype=np.float32)
    b_proj = np.asarray(b_proj, dtype=np.float32)

    if "nc" not in _CACHE:
        _CACHE["nc"] = _build()
    nc = _CACHE["nc"]

    in_maps = _host_inputs(x, ext_mem, W_attn, b_attn, W_proj, b_proj)

    # rare transient corruptions have been observed on this transport; run
    # twice and accept only a reproduced result (third run breaks a tie)
    prev = None
    for attempt in range(4):
        res = run_bass_kernel_spmd(nc, in_maps, list(range(8)))
        cur = [np.asarray(res.results[c]["out"]) for c in range(8)]
        if prev is not None and all(
                np.array_equal(prev[c], cur[c]) for c in range(8)):
            break
        prev = cur

    # v-bias folds through the projection: softmax weights sum to 1, so
    # y = (num/den) + b_v and the output bias becomes b_proj + b_v @ W_proj
    b_eff = b_proj + b_attn[2 * C:3 * C].astype(np.float32) @ W_proj
    out = np.empty((B, T, C), dtype=np.float32)
    for b in range(B):
        out[b] = (cur[2 * b].astype(np.float32)
                  + cur[2 * b + 1].astype(np.float32) + b_eff)
    return out
